# revision 7
# baseline (speedup 1.0000x reference)
"""Trainium2 Bass kernel for multi-head attention (B=2, S=2048, D=1024, H=16, DH=64).

Sharding: tensor-parallel over heads. Each of the 8 NeuronCores computes 2 heads
(h0 at SBUF partitions 0-63, h1 at 64-127) and a partial output projection
against its 128-row slice of Wo; the 8 partial [D, B*S] outputs are summed on
the host (the all-reduce).

All data is bf16 (host pre-converts), all matmuls bf16 with fp32 PSUM:
  - keeps the PE HAM activity monitor warm (no f32r keepalive hacks)
  - enables fast weight load (FWL) for 128-col weights
  - halves HBM traffic
Scores for the two heads are computed CONCURRENTLY via PE row-tiling: K=64
matmuls at tile_position (0,0) and (64,0) (auto-derived from base partitions)
write the two halves of one [128, 2*512] PSUM tile, which one FD=1024 ScalarE
exp converts to probabilities.  ScalarE exp (~127us) is the pipeline pacer;
QKV projections, V transposes and the output projection are interleaved into
the PE slack of the attention phase.

Softmax is computed without max-subtraction (scores ~ N(0,1), fp32 exp safe);
an all-ones column appended to V produces the softmax denominator in PSUM row
64 of each attention accumulator, which then rescales the 64 value rows.
"""

import os
import sys
import types

import numpy as np
import ml_dtypes

B, S, D, H, DH = 2, 2048, 1024, 16, 64
N_CORES = 8
HPC = H // N_CORES          # heads per core = 2
E2 = HPC * DH               # concat head dim per core = 128
T = B * S                   # tokens = 4096
KT = D // 128               # contraction tiles for projections = 8
SBK = 512                   # free-dim block (tokens)
NNB = T // SBK              # projection n-blocks = 8
NSB = S // SBK              # s-blocks per batch = 4
NTB = S // 128              # t-tiles per batch = 16
VW = 2 * (DH + 1)           # vv row width per t-tile = 130 (V_h0|1|V_h1|1)

_STATE = {}


def _ensure_profile_shim():
    """bass_utils wants antenv.axon_hooks for trace=True; this image lacks it."""
    try:
        import antenv.axon_hooks  # noqa: F401
        return
    except ImportError:
        pass
    import antenv
    hook = None
    try:
        from trn_agent_boot.trn_boot import _ntff_profile_via_ctypes
        hook = _ntff_profile_via_ctypes("/opt/axon/libaxon_pjrt.so")
    except Exception:
        hook = None
    mod = types.ModuleType("antenv.axon_hooks")
    mod.get_axon_ntff_profile_hook = lambda: hook
    mod.set_axon_ntff_profile_hook = lambda h: None
    sys.modules["antenv.axon_hooks"] = mod
    antenv.axon_hooks = mod


def _build():
    if "nc" in _STATE:
        return _STATE["nc"]

    import concourse.tile as tile
    from concourse import bacc, mybir
    from concourse.masks import make_identity
    from contextlib import ExitStack

    f32 = mybir.dt.float32
    bf16 = mybir.dt.bfloat16
    Exp = mybir.ActivationFunctionType.Exp

    nc = bacc.Bacc("TRN2", target_bir_lowering=False, debug=False,
                   num_devices=N_CORES)
    xt = nc.declare_dram_parameter("xt", [D, T], bf16, isOutput=False)
    wq = nc.declare_dram_parameter("wq", [D, E2], bf16, isOutput=False)
    wk = nc.declare_dram_parameter("wk", [D, E2], bf16, isOutput=False)
    wv = nc.declare_dram_parameter("wv", [D, E2], bf16, isOutput=False)
    wo = nc.declare_dram_parameter("wo", [E2, D], bf16, isOutput=False)
    outT = nc.declare_dram_parameter("outT", [D, T], bf16, isOutput=True)
    dbg = bool(os.environ.get("BASS_V2_DEBUG"))
    if dbg:
        at_dbg = nc.declare_dram_parameter("at_dbg", [128, T], bf16,
                                           isOutput=True)
        qt_dbg = nc.declare_dram_parameter("qt_dbg", [128, T], bf16,
                                           isOutput=True)
        kt_dbg = nc.declare_dram_parameter("kt_dbg", [128, T], bf16,
                                           isOutput=True)
        vv_dbg = nc.declare_dram_parameter("vv_dbg", [128, (T // 128) * VW],
                                           bf16, isOutput=True)

    with tile.TileContext(nc) as tc, ExitStack() as ctx:
        const = ctx.enter_context(tc.tile_pool(name="const", bufs=1))
        big = ctx.enter_context(tc.tile_pool(name="big", bufs=1))

        qt = big.tile([128, T], bf16, tag="qt")        # Q^T  [2h*64, tok]
        kt = big.tile([128, T], bf16, tag="kt")        # K^T
        vv = big.tile([128, T // 128, VW], bf16, tag="vv")  # V' per t-tile
        at = big.tile([128, T], bf16, tag="at")        # attn^T concat [e2, tok]
        wq_sb = big.tile([128, KT, E2], bf16, tag="wq")
        wk_sb = big.tile([128, KT, E2], bf16, tag="wk")
        wv_sb = big.tile([128, KT, E2], bf16, tag="wv")
        wo_sb = big.tile([128, D], bf16, tag="wo")

        ident = const.tile([128, 128], bf16, tag="ident")
        ones_f32 = const.tile([128, 64], f32, tag="ones_f32")
        kab1 = const.tile([128, 1], bf16, tag="kab1")
        kab2 = const.tile([128, SBK], bf16, tag="kab2")
        make_identity(nc, ident[:])
        nc.vector.memset(ones_f32[:], 1.0)
        nc.vector.memset(kab1[:], 1.0)
        nc.vector.memset(kab2[:], 1.0)
        # denominator columns of V' (col 64 for head0, col 129 for head1)
        nc.vector.tensor_copy(vv[:, :, DH], ones_f32[:, 0:T // 128])
        nc.vector.tensor_copy(vv[:, :, DH + 1 + DH], ones_f32[:, 0:T // 128])

        # weights ride the ScalarE-issued DMA queue so they don't serialize
        # behind the xt activations on the sync queue
        kblocked = lambda ap: ap.rearrange("(ko ki) e -> ki ko e", ki=128)
        nc.scalar.dma_start(out=wq_sb[:], in_=kblocked(wq))
        nc.scalar.dma_start(out=wk_sb[:], in_=kblocked(wk))
        nc.scalar.dma_start(out=wv_sb[:], in_=kblocked(wv))
        nc.scalar.dma_start(out=wo_sb[:], in_=wo[:])

        xt_blk = xt.rearrange("(ko ki) t -> ki ko t", ki=128)

        xtp = ctx.enter_context(tc.tile_pool(name="xtp", bufs=3))
        vtp = ctx.enter_context(tc.tile_pool(name="vtp", bufs=2))
        punp = ctx.enter_context(tc.tile_pool(name="punp", bufs=4))
        nrm = ctx.enter_context(tc.tile_pool(name="nrm", bufs=4))
        otp = ctx.enter_context(tc.tile_pool(name="otp", bufs=6))
        psc = ctx.enter_context(tc.tile_pool(name="psc", bufs=2, space="PSUM"))
        pat = ctx.enter_context(tc.tile_pool(name="pat", bufs=2, space="PSUM"))
        ppj = ctx.enter_context(tc.tile_pool(name="ppj", bufs=2, space="PSUM"))

        # initial PE warmup burst: trip the HAM busy detector while the
        # first DMAs are in flight
        pkw = ppj.tile([128, SBK], f32, tag="pj", name="kw")
        for _ in range(6):
            nc.tensor.matmul(pkw[0:1, :], kab1[:], kab2[:], start=True,
                             stop=True)

        xtis = {}

        def emit_proj_dma(n):
            xti = xtp.tile([128, KT, SBK], bf16, tag="xt", name=f"xti_{n}")
            xtis[n] = xti
            for c in range(4):
                nc.sync.dma_start(
                    out=xti[:, 2 * c:2 * c + 2, :],
                    in_=xt_blk[:, 2 * c:2 * c + 2, n * SBK:(n + 1) * SBK])

        def emit_proj_pass(n, which):
            """One projection pass (q, k or v) for n-block n."""
            xti = xtis[n]
            ps = ppj.tile([128, SBK], f32, tag="pj", name=f"ps_{which}_{n}")
            w_sb = {"q": wq_sb, "k": wk_sb, "v": wv_sb}[which]
            for k in range(KT):
                nc.tensor.matmul(ps[:], w_sb[:, k, :], xti[:, k, :],
                                 start=(k == 0), stop=(k == KT - 1))
            # copy-outs ride ScalarE (it has slack around the exp stream and,
            # unlike the DVE queue, isn't backlogged behind norm chains and
            # output-copy CASTs) so the PSUM slot frees promptly for the PE
            if which == "q":
                nc.scalar.copy(qt[:, n * SBK:(n + 1) * SBK], ps[:])
            elif which == "k":
                nc.scalar.copy(kt[:, n * SBK:(n + 1) * SBK], ps[:])
            else:
                vt = vtp.tile([128, SBK], bf16, tag="vt", name=f"vt_{n}")
                nc.scalar.copy(vt[:], ps[:])
                # all 4 transposes of this n-block share one PSUM slot;
                # one fused scatter moves them into the interleaved vv layout
                ptr = ppj.tile([128, 4, 128], bf16, tag="pj",
                               name=f"tr_{n}")
                for j in range(SBK // 128):
                    nc.tensor.transpose(ptr[:, j, :],
                                        vt[:, j * 128:(j + 1) * 128],
                                        ident[:])
                t0 = n * (SBK // 128)
                dst = vv[:, t0:t0 + 4, :].rearrange(
                    "p a (h eo) -> p a h eo", h=2)[:, :, :, 0:DH]
                src = ptr.rearrange("p a (h e) -> p a h e", h=2)
                nc.vector.tensor_copy(dst, src)

        def emit_scores_exp(b, sj, ti):
            """Scores (both heads, row-packed) -> exp.  Returns pun."""
            tsl = slice(b * S + ti * 128, b * S + (ti + 1) * 128)
            ssl = slice(b * S + sj * SBK, b * S + (sj + 1) * SBK)
            pss = psc.tile([128, 2, SBK], f32, tag="sc",
                           name=f"pss_{b}_{sj}_{ti}")
            nc.tensor.matmul(pss[:, 0, :], kt[0:64, tsl], qt[0:64, ssl],
                             start=True, stop=True)
            nc.tensor.matmul(pss[:, 1, :], kt[64:128, tsl], qt[64:128, ssl],
                             start=True, stop=True)
            pun = punp.tile([128, 2, SBK], bf16, tag="pun",
                            name=f"pun_{b}_{sj}_{ti}")
            nc.scalar.activation(pun[:], pss[:], Exp, scale=0.125)
            return pun

        def emit_attn(b, sj, ti, pun, psat0, psat1):
            st, sp = (ti == 0), (ti == NTB - 1)
            tt = b * NTB + ti
            nc.tensor.matmul(psat0[:], vv[:, tt, 0:DH + 1], pun[:, 0, :],
                             start=st, stop=sp)
            nc.tensor.matmul(psat1[:], vv[:, tt, DH + 1:VW], pun[:, 1, :],
                             start=st, stop=sp)

        def emit_norms(b, sj, psat0, psat1):
            # both heads' chains interleaved: the two asb copies go first so
            # both psat PSUM slots free immediately (the next slot's first
            # attn matmuls wait on them), and the gpsimd broadcasts overlap
            # the DVE work of the other head.
            ssl = slice(b * S + sj * SBK, b * S + (sj + 1) * SBK)
            asb, den, recip, bcast = [], [], [], []
            for h, psat in ((0, psat0), (1, psat1)):
                asb.append(nrm.tile([DH + 1, SBK], f32, tag=f"asb{h}",
                                    name=f"asb_{b}_{sj}_{h}"))
                nc.vector.tensor_copy(asb[h][:], psat[:])
            for h in range(2):
                # den must sit on a base-partition-0 tile:
                # reciprocal_approx_fast misreads partition-offset inputs.
                den.append(nrm.tile([1, SBK], f32, tag=f"den{h}",
                                    name=f"den_{b}_{sj}_{h}"))
                nc.vector.tensor_copy(den[h][:], asb[h][DH:DH + 1, :])
                recip.append(nrm.tile([1, SBK], f32, tag=f"recip{h}",
                                      name=f"recip_{b}_{sj}_{h}"))
                nc.vector.reciprocal_approx_fast(recip[h][:], den[h][:])
                bcast.append(nrm.tile([DH, SBK], f32, tag=f"bcast{h}",
                                      name=f"bcast_{b}_{sj}_{h}"))
                nc.gpsimd.partition_broadcast(bcast[h][:], recip[h][:])
            for h in range(2):
                nc.vector.tensor_mul(at[h * DH:(h + 1) * DH, ssl],
                                     asb[h][0:DH, :], bcast[h][:])

        def emit_outproj(b, sj, tail=False, half=None):
            sn = b * NSB + sj
            ssl = slice(sn * SBK, (sn + 1) * SBK)
            dos = range(D // 128) if half is None else \
                range(half * 4, half * 4 + 4)
            for do in dos:
                po = ppj.tile([128, SBK], f32, tag="pj",
                              name=f"po_{b}_{sj}_{do}")
                nc.tensor.matmul(po[:], wo_sb[:, do * 128:(do + 1) * 128],
                                 at[:, ssl], start=True, stop=True)
                ot = otp.tile([128, SBK], bf16, tag="ot",
                              name=f"ot_{b}_{sj}_{do}")
                if tail and do % 2 == 0:
                    nc.scalar.copy(ot[:], po[:])
                else:
                    nc.vector.tensor_copy(ot[:], po[:])
                nc.sync.dma_start(
                    out=outT[do * 128:(do + 1) * 128, ssl], in_=ot[:])

        # ---- emission schedule ----
        # Filler work (remaining projection passes, output projections) is
        # interleaved into the attention slots; the ScalarE exp stream paces
        # the pipeline and the PE consumes filler during its slack.
        # Lead-in: first n-block only, then pipeline the rest inside the
        # attention slots.  A proj block n must be FULLY emitted before any
        # ti >= 4n of its batch (PE queue is in-order; attn(ti) waiting on
        # vv ahead of the v-pass that produces it would deadlock).
        emit_proj_dma(0)
        emit_proj_dma(1)
        emit_proj_dma(2)
        emit_proj_pass(0, "q")
        emit_proj_pass(0, "k")

        def PP(n):
            return [("pp", n, "q"), ("pp", n, "k"), ("pp", n, "v"),
                    ("pd", n + 2)]

        # plan[(b, sj)] = {ti: [items emitted after that ti]}
        def spread(items, tis):
            return {ti: [it] for ti, it in zip(tis, items)}

        plan = {
            (0, 0): spread(PP(1) + PP(2) + PP(3),
                           [2, 3, 4, 5, 6, 7, 8, 9, 10, 11, 12, 13]),
            (0, 1): spread(PP(4), [0, 1, 2, 3]),
            (0, 2): spread(PP(5), [0, 1, 2, 3]),
            (0, 3): spread(PP(6)[:3], [0, 1, 2]),
            (1, 0): spread(PP(7)[:3] + [("op", 0, 0, 0), ("op", 0, 0, 1)],
                           [0, 1, 2, 8, 11]),
            (1, 1): spread([("op", 0, 1, 0), ("op", 0, 1, 1),
                            ("op", 0, 2, 0), ("op", 0, 2, 1)], [2, 5, 9, 12]),
            (1, 2): spread([("op", 0, 3, 0), ("op", 0, 3, 1),
                            ("op", 1, 0, 0), ("op", 1, 0, 1)], [2, 5, 9, 12]),
            (1, 3): spread([("op", 1, 1, 0), ("op", 1, 1, 1),
                            ("op", 1, 2, 0), ("op", 1, 2, 1)], [2, 5, 9, 12]),
        }
        # the v-pass of block 0 must slip in between scores(ti1) and
        # attn(ti0) — after the first exp is already in flight, but before
        # the first attn (which reads vv) enters the PE queue
        preplan = {(0, 0): {1: [("pp", 0, "v")]}}

        for b in range(B):
            for sj in range(NSB):
                psat0 = pat.tile([DH + 1, SBK], f32, tag="at",
                                 name=f"psat0_{b}_{sj}")
                psat1 = pat.tile([DH + 1, SBK], f32, tag="at",
                                 name=f"psat1_{b}_{sj}")
                slot = plan.get((b, sj), {})
                # software pipeline: attn for ti-1 is emitted after
                # scores/exp for ti, so attn never waits at the head of the
                # PE queue for an exp still in flight (and queued filler work
                # behind it is never blocked).
                preslot = preplan.get((b, sj), {})
                prev_pun = None
                for ti in range(NTB):
                    pun = emit_scores_exp(b, sj, ti)
                    for item in preslot.get(ti, []):
                        emit_proj_pass(item[1], item[2])
                    if prev_pun is not None:
                        emit_attn(b, sj, ti - 1, prev_pun, psat0, psat1)
                    prev_pun = pun
                    for item in slot.get(ti, []):
                        if item[0] == "pp":
                            emit_proj_pass(item[1], item[2])
                        elif item[0] == "pd":
                            if item[1] < NNB:
                                emit_proj_dma(item[1])
                        else:
                            emit_outproj(item[1], item[2], half=item[3])
                emit_attn(b, sj, NTB - 1, prev_pun, psat0, psat1)
                if b == B - 1 and sj == NSB - 1:
                    # dependency-free keepalive MMs run during the final
                    # normalization chain so HAM stays at full clock for the
                    # tail output projection
                    pka = ppj.tile([128, SBK], f32, tag="pj", name="ka_tail")
                    for _ in range(26):
                        nc.tensor.matmul(pka[0:1, :], kab1[:], kab2[:],
                                         start=True, stop=True)
                emit_norms(b, sj, psat0, psat1)
        emit_outproj(1, 3, tail=True)
        if dbg:
            nc.sync.dma_start(out=at_dbg[:], in_=at[:])
            nc.sync.dma_start(out=qt_dbg[:], in_=qt[:])
            nc.sync.dma_start(out=kt_dbg[:], in_=kt[:])
            nc.sync.dma_start(out=vv_dbg[:],
                              in_=vv.rearrange("p a b -> p (a b)"))

    nc.compile()
    _STATE["nc"] = nc
    return nc


def _prep_inputs(hidden_state, Wq, Wk, Wv, Wo):
    bf = ml_dtypes.bfloat16
    xt = np.ascontiguousarray(
        np.asarray(hidden_state, dtype=np.float32).reshape(T, D).T).astype(bf)
    in_maps = []
    for c in range(N_CORES):
        h0 = c * HPC
        wq_c = np.ascontiguousarray(
            np.asarray(Wq[h0:h0 + HPC], dtype=np.float32)
            .transpose(1, 0, 2).reshape(D, E2)).astype(bf)
        wk_c = np.ascontiguousarray(
            np.asarray(Wk[h0:h0 + HPC], dtype=np.float32)
            .transpose(1, 0, 2).reshape(D, E2)).astype(bf)
        wv_c = np.ascontiguousarray(
            np.asarray(Wv[h0:h0 + HPC], dtype=np.float32)
            .transpose(1, 0, 2).reshape(D, E2)).astype(bf)
        wo_c = np.ascontiguousarray(
            np.asarray(Wo[c * E2:(c + 1) * E2], dtype=np.float32)).astype(bf)
        in_maps.append({"xt": xt, "wq": wq_c, "wk": wk_c, "wv": wv_c,
                        "wo": wo_c})
    return in_maps


def _run(in_maps, trace=False):
    from concourse.bass_utils import run_bass_kernel_spmd
    if trace:
        _ensure_profile_shim()
    nc = _build()
    if trace:
        # Warm the device (clocks, NEFF residency) so the traced run
        # measures steady-state performance.
        run_bass_kernel_spmd(nc, in_maps, list(range(N_CORES)), trace=False)
    return run_bass_kernel_spmd(nc, in_maps, list(range(N_CORES)), trace=trace)


def kernel(hidden_state, Wq, Wk, Wv, Wo):
    in_maps = _prep_inputs(hidden_state, Wq, Wk, Wv, Wo)
    trace = bool(os.environ.get("BASS_KERNEL_TRACE"))
    res = _run(in_maps, trace=trace)
    if trace and res.exec_time_ns is not None:
        print(f"HW exec time: {res.exec_time_ns} ns")
    acc = np.zeros((D, T), dtype=np.float64)
    for c in range(N_CORES):
        acc += res.results[c]["outT"].astype(np.float64)
    return np.ascontiguousarray(acc.T.reshape(B, S, D)).astype(np.float32)
